# revision 9
# baseline (speedup 1.0000x reference)
"""CronRoot (sqrt-N block-sparse causal) multihead attention on 8 trn2 cores.

v2: all-bf16 matmuls (1 cycle/row on the PE at any free size), fully
SBUF-resident q/k/v (no DRAM staging), block-diagonal packed summary-key
and summary-value stationaries (one matmul serves both heads of a pair),
folded biases (v-bias + out-bias fold into a host-precomputed output bias;
q/k biases ride the PSUM-evacuation activations), transposed P3 output so
its bias is per-partition, and softmax bookkeeping (denominator l, 1/l
broadcast) done with minimal ones-matmuls.

Sharding: sequence-parallel. Each core owns 8 of the 64 blocks (512
positions) for all batches/heads; summary k/v recomputed per-core from the
256 summary rows of x (no collectives).

Engine split per (b, head-pair) attention instance:
  PE: 8 local-score mm, 1 summary-score mm (block-diag ksd), 3 denominator
      mm, 1 bcast mm, 9 AV mm (block-diag vsd + 8 local).
  Scalar: exp(local [128,1024]), exp(summary [128,512]).
  GpSimd: local mask multiply. DVE: summary mask multiply, reciprocal,
  final (attn*1/l) -> bf16 attnT.
"""

import os
import numpy as np
import ml_dtypes
from contextlib import ExitStack

V2_NOPOOL = bool(int(os.environ.get("V2_NOPOOL", "0")))
V2_NODUP = bool(int(os.environ.get("V2_NODUP", "0")))
V2_SPLITEXP = bool(int(os.environ.get("V2_SPLITEXP", "0")))

import concourse.bass as bass  # noqa: F401
import concourse.tile as tile
from concourse import bacc, mybir
from concourse.bass_utils import run_bass_kernel_spmd

F32 = mybir.dt.float32
BF16 = mybir.dt.bfloat16
AF = mybir.ActivationFunctionType

B, S, D = 4, 4096, 1024
H, HD = 16, 64
BLK = 64                 # block size (= sqrt(S))
NB = S // BLK            # 64 blocks
NCORES = 8
SC = S // NCORES         # 512 seq positions per core
BPC = NB // NCORES       # 8 blocks per core
TC = B * SC              # 2048 (b-major) t columns per core
NSUM = B * NB            # 256 summary positions (b-major)
SCALE = 1.0 / np.sqrt(HD)


def build_nc(repeat=1, phases=(1, 2, 3)):
    nc = bacc.Bacc("TRN2", target_bir_lowering=False, debug=False,
                   num_devices=NCORES)

    xT = nc.dram_tensor("xT", [D, TC], BF16, kind="ExternalInput").ap()
    xsT = nc.dram_tensor("xsT", [D, NSUM], BF16, kind="ExternalInput").ap()
    wqkT = nc.dram_tensor("wqkT", [D, 2 * D], BF16, kind="ExternalInput").ap()
    wvT = nc.dram_tensor("wvT", [D, D], BF16, kind="ExternalInput").ap()
    biT = nc.dram_tensor("biT", [128, 16], F32, kind="ExternalInput").ap()
    woT = nc.dram_tensor("woT", [D, D], BF16, kind="ExternalInput").ap()
    boT = nc.dram_tensor("boT", [128, 8], F32, kind="ExternalInput").ap()
    cst = nc.dram_tensor("cst", [128, 6], BF16, kind="ExternalInput").ap()
    dTt = nc.dram_tensor("dTt", [2, 128], BF16, kind="ExternalInput").ap()
    mloc2 = nc.dram_tensor("mloc2", [128, 1024], BF16, kind="ExternalInput").ap()
    msum2 = nc.dram_tensor("msum2", [128, SC], BF16, kind="ExternalInput").ap()
    outT = nc.dram_tensor("outT", [D, TC], BF16, kind="ExternalOutput").ap()

    with tile.TileContext(nc) as tc_:
      for _rep in range(repeat):
       with ExitStack() as ctx:
        pp = ctx.enter_context(tc_.tile_pool(name="persist", bufs=1))
        qT = pp.tile([128, 8, TC], BF16, tag="qT")
        kT = pp.tile([128, 8, TC], BF16, tag="kT")
        v_sb = pp.tile([128, 16, 16, HD], BF16, tag="v")     # (tcn, h, d)
        ksd = pp.tile([128, 32, 128], BF16, tag="ksd")       # (hp*4+b) diag
        vsd = pp.tile([128, 32, 128], BF16, tag="vsd")
        attnT = pp.tile([128, 8, TC], BF16, tag="attnT")
        biT_sb = pp.tile([128, 16], F32, tag="biT")
        nc.sync.dma_start(biT_sb[:], biT[:])
        boT_sb = pp.tile([128, 8], F32, tag="boT")
        nc.sync.dma_start(boT_sb[:], boT[:])
        cst_sb = pp.tile([128, 6], BF16, tag="cst")
        nc.sync.dma_start(cst_sb[:], cst[:])
        dT_sb = pp.tile([2, 128], BF16, tag="dT")
        nc.sync.dma_start(dT_sb[:], dTt[:])
        mloc2_sb = pp.tile([128, 1024], BF16, tag="mloc2")
        nc.sync.dma_start(mloc2_sb[:], mloc2[:])
        msum2_sb = pp.tile([128, SC], BF16, tag="msum2")
        nc.sync.dma_start(msum2_sb[:], msum2[:])
        nc.vector.memset(ksd[:], 0.0)
        nc.vector.memset(vsd[:], 0.0)

        # ---------------- P1: projections ----------------
        if 1 in phases:
         with tc_.tile_pool(name="p1", bufs=2) as p1, \
              tc_.tile_pool(name="ps1", bufs=2, space="PSUM") as ps1:
             xT_sb = p1.tile([128, 8, TC], BF16, tag="xT", bufs=1)
             for dc in range(8):
                 nc.sync.dma_start(xT_sb[:, dc, :], xT[dc * 128:(dc + 1) * 128, :])
             xsT_sb = p1.tile([128, 8, NSUM], BF16, tag="xsT", bufs=1)
             for dc in range(8):
                 nc.sync.dma_start(xsT_sb[:, dc, :],
                                   xsT[dc * 128:(dc + 1) * 128, :])

             for vh in range(2):
                 for hp in range(4 * vh, 4 * vh + 4):
                     # q chunk
                     wq_sb = p1.tile([128, 8, 128], BF16, tag="w_sb")
                     for dc in range(8):
                         nc.sync.dma_start(
                             wq_sb[:, dc, :],
                             wqkT[dc * 128:(dc + 1) * 128,
                                  hp * 128:(hp + 1) * 128])
                     for tt in range(4):
                         ps_qk = ps1.tile([128, 512], F32, tag="ps_qk")
                         for dc in range(8):
                             nc.tensor.matmul(
                                 ps_qk[:], wq_sb[:, dc, :],
                                 xT_sb[:, dc, tt * 512:(tt + 1) * 512],
                                 start=(dc == 0), stop=(dc == 7))
                         nc.scalar.activation(
                             qT[:, hp, tt * 512:(tt + 1) * 512], ps_qk[:],
                             AF.Identity, bias=biT_sb[:, hp:hp + 1])
                     # k chunk (+ summary keys into block-diag ksd)
                     wk_sb = p1.tile([128, 8, 128], BF16, tag="w_sb")
                     for dc in range(8):
                         nc.sync.dma_start(
                             wk_sb[:, dc, :],
                             wqkT[dc * 128:(dc + 1) * 128,
                                  D + hp * 128: D + (hp + 1) * 128])
                     for tt in range(4):
                         ps_qk = ps1.tile([128, 512], F32, tag="ps_qk")
                         for dc in range(8):
                             nc.tensor.matmul(
                                 ps_qk[:], wk_sb[:, dc, :],
                                 xT_sb[:, dc, tt * 512:(tt + 1) * 512],
                                 start=(dc == 0), stop=(dc == 7))
                         nc.scalar.activation(
                             kT[:, hp, tt * 512:(tt + 1) * 512], ps_qk[:],
                             AF.Identity, bias=biT_sb[:, hp + 8:hp + 9])
                     ps_ks = ps1.tile([128, 4, 64], F32, tag="ps_ks")
                     for dc in range(8):
                         nc.tensor.matmul(ps_ks[:], wk_sb[:, dc, :],
                                          xsT_sb[:, dc, :],
                                          start=(dc == 0), stop=(dc == 7))
                     nc.scalar.activation(
                         ksd[0:64, hp * 4:hp * 4 + 4, 0:64], ps_ks[0:64],
                         AF.Identity, bias=biT_sb[0:64, hp + 8:hp + 9])
                     nc.scalar.activation(
                         ksd[64:128, hp * 4:hp * 4 + 4, 64:128], ps_ks[64:128],
                         AF.Identity, bias=biT_sb[64:128, hp + 8:hp + 9])

                 # v features for this half (heads 8*vh .. 8*vh+8)
                 wv_sb = p1.tile([128, 8, 512], BF16, tag="wv_sb", bufs=1)
                 for dc in range(8):
                     nc.sync.dma_start(
                         wv_sb[:, dc, :],
                         wvT[dc * 128:(dc + 1) * 128,
                             vh * 512:(vh + 1) * 512])
                 for tcn in range(16):
                     ps_v = ps1.tile([128, 512], F32, tag="ps_v")
                     for dc in range(8):
                         nc.tensor.matmul(
                             ps_v[:],
                             xT_sb[:, dc, tcn * 128:(tcn + 1) * 128],
                             wv_sb[:, dc, :],
                             start=(dc == 0), stop=(dc == 7))
                     nc.scalar.copy(
                         v_sb[:, tcn, vh * 8:(vh + 1) * 8, :], ps_v[:])
                 # summary v -> vs2 (with duplicated partition halves)
                 vs2 = p1.tile([128, 4, 8, HD], BF16, tag="vs2", bufs=1)
                 for sch in range(2):
                     ps_vs = ps1.tile([128, 512], F32, tag="ps_vs")
                     for dc in range(8):
                         nc.tensor.matmul(
                             ps_vs[:],
                             xsT_sb[:, dc, sch * 128:(sch + 1) * 128],
                             wv_sb[:, dc, :],
                             start=(dc == 0), stop=(dc == 7))
                     nc.scalar.copy(vs2[0:64, 2 * sch, :, :], ps_vs[0:64])
                     nc.scalar.copy(vs2[64:128, 2 * sch + 1, :, :],
                                    ps_vs[64:128])
                     if V2_NODUP:
                         vtmp = p1.tile([128, 8, HD], BF16, tag="vtmp")
                         nc.scalar.copy(vtmp[0:64, :, :], ps_vs[0:64])
                         nc.scalar.copy(vtmp[64:128, :, :], ps_vs[64:128])
                         nc.sync.dma_start(vs2[64:128, 2 * sch, :, :],
                                           vtmp[0:64, :, :])
                         nc.sync.dma_start(vs2[0:64, 2 * sch + 1, :, :],
                                           vtmp[64:128, :, :])
                     else:
                         nc.sync.dma_start(vs2[64:128, 2 * sch, :, :],
                                           vs2[0:64, 2 * sch, :, :])
                         nc.sync.dma_start(vs2[0:64, 2 * sch + 1, :, :],
                                           vs2[64:128, 2 * sch + 1, :, :])
                 # block-diag summary-v stationaries for this vh's head pairs
                 for hp in range(4 * vh, 4 * vh + 4):
                     hl = 2 * hp - 8 * vh        # head index within vs2 cols
                     nc.vector.tensor_copy(
                         vsd[0:64, hp * 4:hp * 4 + 4, 0:64],
                         vs2[0:64, :, hl, :])
                     nc.vector.tensor_copy(
                         vsd[64:128, hp * 4:hp * 4 + 4, 64:128],
                         vs2[64:128, :, hl + 1, :])

        # ---------------- P2: attention ----------------
        if 2 in phases:
         with tc_.tile_pool(name="p2", bufs=2) as p2, \
              tc_.tile_pool(name="ps_sl", bufs=1, space="PSUM") as ps_sl, \
              tc_.tile_pool(name="ps_ss", bufs=1, space="PSUM") as ps_ss, \
              tc_.tile_pool(name="ps_l", bufs=1, space="PSUM") as ps_l, \
              tc_.tile_pool(name="ps_bc", bufs=1, space="PSUM") as ps_bc, \
              tc_.tile_pool(name="ps_av", bufs=2, space="PSUM") as ps_av:
            for hp in range(8):
                for b in range(B):
                    c0 = b * 512
                    s_loc = ps_sl.tile([128, 1024], F32, tag="s_loc")
                    for hh in range(2):
                        p0 = hh * 64
                        for p4 in range(4):
                            cq = c0 + p4 * 128
                            nc.tensor.matmul(
                                s_loc[:, hh * 512 + p4 * 128:
                                      hh * 512 + (p4 + 1) * 128],
                                kT[p0:p0 + 64, hp, cq:cq + 128],
                                qT[p0:p0 + 64, hp, cq:cq + 128],
                                start=True, stop=True)
                    s_sum = ps_ss.tile([128, 512], F32, tag="s_sum")
                    nc.tensor.matmul(s_sum[:], ksd[:, hp * 4 + b, :],
                                     qT[:, hp, c0:c0 + 512],
                                     start=True, stop=True)
                    pml_e = p2.tile([128, 1024], BF16, tag="pml_e")
                    if V2_SPLITEXP:
                        nc.scalar.activation(pml_e[:, 0:512], s_loc[:, 0:512],
                                             AF.Exp, scale=SCALE)
                        nc.scalar.activation(pml_e[:, 512:1024],
                                             s_loc[:, 512:1024],
                                             AF.Exp, scale=SCALE)
                    else:
                        nc.scalar.activation(pml_e[:], s_loc[:], AF.Exp,
                                             scale=SCALE)
                    pms_e = p2.tile([128, 512], BF16, tag="pms_e")
                    nc.scalar.activation(pms_e[:], s_sum[:], AF.Exp,
                                         scale=SCALE)
                    pml = p2.tile([128, 1024], BF16, tag="pml")
                    mask_eng = nc.vector if V2_NOPOOL else nc.gpsimd
                    mask_eng.tensor_mul(pml[:], pml_e[:], mloc2_sb[:])
                    pms = p2.tile([128, 512], BF16, tag="pms")
                    nc.vector.tensor_mul(pms[:], pms_e[:], msum2_sb[:])
                    l_ps = ps_l.tile([2, 512], F32, tag="l")
                    nc.tensor.matmul(l_ps[:], cst_sb[:, 0:2], pms[:],
                                     start=True, stop=False)
                    nc.tensor.matmul(l_ps[:], cst_sb[:, 2:4], pml[:, 0:512],
                                     start=False, stop=False)
                    nc.tensor.matmul(l_ps[:], cst_sb[:, 4:6], pml[:, 512:1024],
                                     start=False, stop=True)
                    r2 = p2.tile([2, 512], BF16, tag="r2")
                    with nc.allow_low_precision(reason="bf16 softmax recip"):
                        nc.vector.reciprocal(r2[:], l_ps[:])
                    bc = ps_bc.tile([128, 512], F32, tag="bc")
                    nc.tensor.matmul(bc[:], dT_sb[:], r2[:],
                                     start=True, stop=True)
                    av = ps_av.tile([128, 512], F32, tag="av")
                    nc.tensor.matmul(av[:], vsd[:, hp * 4 + b, :], pms[:],
                                     start=True, stop=False,
                                     skip_group_check=True)
                    for hh in range(2):
                        for p4 in range(4):
                            nc.tensor.matmul(
                                av[hh * 64:hh * 64 + 64,
                                   p4 * 128:(p4 + 1) * 128],
                                v_sb[:, b * 4 + p4, 2 * hp + hh, :],
                                pml[:, hh * 512 + p4 * 128:
                                    hh * 512 + (p4 + 1) * 128],
                                start=False, stop=(p4 == 3),
                                skip_group_check=True)
                    av_sb = p2.tile([128, 512], BF16, tag="av_sb")
                    nc.scalar.copy(av_sb[:], av[:])
                    nc.vector.tensor_mul(attnT[:, hp, c0:c0 + 512],
                                         av_sb[:], bc[:])

        # ---------------- P3: output projection (transposed out) ----------
        if 3 in phases:
         with tc_.tile_pool(name="p3", bufs=2) as p3, \
              tc_.tile_pool(name="ps3", bufs=2, space="PSUM") as ps3:
            woT_sb = p3.tile([128, 8, D], BF16, tag="woT", bufs=1)
            for dc in range(8):
                nc.sync.dma_start(woT_sb[:, dc, :],
                                  woT[dc * 128:(dc + 1) * 128, :])
            for oc in range(8):
                for tt in range(4):
                    ps_o = ps3.tile([128, 512], F32, tag="ps_o")
                    for hp in range(8):
                        nc.tensor.matmul(
                            ps_o[:],
                            woT_sb[:, hp, oc * 128:(oc + 1) * 128],
                            attnT[:, hp, tt * 512:(tt + 1) * 512],
                            start=(hp == 0), stop=(hp == 7))
                    o_sb = p3.tile([128, 512], BF16, tag="o_sb")
                    nc.scalar.activation(o_sb[:], ps_o[:], AF.Identity,
                                         bias=boT_sb[:, oc:oc + 1])
                    nc.sync.dma_start(
                        outT[oc * 128:(oc + 1) * 128,
                             tt * 512:(tt + 1) * 512], o_sb[:])

    nc.compile()
    return nc


def make_in_maps(x, in_proj_weight, in_proj_bias, out_proj_weight,
                 out_proj_bias):
    f32, bf16 = np.float32, ml_dtypes.bfloat16
    x = np.asarray(x, f32)
    wiT = np.asarray(in_proj_weight, f32).T              # [D, 3D]
    wqkT = np.ascontiguousarray(wiT[:, :2 * D]).astype(bf16)
    wvT = np.ascontiguousarray(wiT[:, 2 * D:]).astype(bf16)
    bi = np.asarray(in_proj_bias, f32)
    biT = np.ascontiguousarray(bi[:2 * D].reshape(16, 128).T)
    wo = np.asarray(out_proj_weight, f32)
    woT = np.ascontiguousarray(wo.T).astype(bf16)
    bop = wo @ bi[2 * D:] + np.asarray(out_proj_bias, f32)
    boT = np.ascontiguousarray(bop.reshape(8, 128).T)

    p = np.arange(128)
    cst = np.zeros((128, 6), f32)
    cst[:, 0] = p < 64
    cst[:, 1] = p >= 64
    cst[:, 2] = 1.0
    cst[:, 5] = 1.0
    cst = cst.astype(bf16)
    dTt = np.zeros((2, 128), f32)
    dTt[0, :64] = 1.0
    dTt[1, 64:] = 1.0
    dTt = dTt.astype(bf16)

    k2 = np.arange(128)[:, None]
    q = np.arange(SC)[None, :]
    mloc = (((k2 // 64) == ((q // 64) % 2)) & ((q % 64) >= (k2 % 64)))
    mloc2 = np.tile(mloc.astype(f32), (1, 2)).astype(bf16)

    xs = x[:, BLK - 1::BLK, :]                           # [B, 64, D]
    xsT = np.ascontiguousarray(
        xs.transpose(2, 0, 1).reshape(D, NSUM)).astype(bf16)

    m = np.arange(64)[:, None]
    in_maps = []
    for c in range(NCORES):
        xc = x[:, c * SC:(c + 1) * SC, :]                # [B, 512, D]
        xTc = np.ascontiguousarray(
            xc.transpose(2, 0, 1).reshape(D, TC)).astype(bf16)
        ms = (m < (c * BPC + (q // 64))).astype(f32)     # [64, 512]
        msum2 = np.concatenate([ms, ms], 0).astype(bf16)
        in_maps.append({
            "xT": xTc, "xsT": xsT, "wqkT": wqkT, "wvT": wvT, "biT": biT,
            "woT": woT, "boT": boT, "cst": cst, "dTt": dTt,
            "mloc2": mloc2, "msum2": msum2,
        })
    return in_maps


_NC_CACHE = []


def kernel(x, in_proj_weight, in_proj_bias, out_proj_weight, out_proj_bias):
    if not _NC_CACHE:
        _NC_CACHE.append(build_nc())
    nc = _NC_CACHE[0]
    in_maps = make_in_maps(x, in_proj_weight, in_proj_bias, out_proj_weight,
                           out_proj_bias)
    res = run_bass_kernel_spmd(nc, in_maps, core_ids=list(range(NCORES)))
    out = np.empty((B, S, D), np.float32)
    for c in range(NCORES):
        oT = np.asarray(res.results[c]["outT"]).astype(np.float32)
        out[:, c * SC:(c + 1) * SC, :] = \
            oT.reshape(D, B, SC).transpose(1, 2, 0)
    return out
